# revision 13
# baseline (speedup 1.0000x reference)
"""Trainium2 Bass kernel: DarkChannelLoss.

Computes -mean(dark_channel(x)) for x [32,3,512,512] f32, where
dark_channel = reflect-pad(7) -> min over channels -> 15x15 sliding-window
min (windows clipped at bottom/right, i.e. +inf padded by 14).

Sharding: pure data parallel over batch, 4 images per NeuronCore x 8 cores.
Each core computes per-partition partial sums of its dark-channel map; the
host combines them into the final scalar mean.

Per-core pipeline (shapes hardcoded):
  load:   3 channels per image as bf16 (SWDGE cast DMA), rows (padded, with
          top/bottom reflection resolved by the DMA row map) on partitions,
          5 row-tiles packed into one [128, 5, 512] tile per channel.
  pass 1: channel-min (bf16), left/right reflect pads via small reversed
          copies, then sliding-min cascade along W (windows 2,4,8,15).
  transpose: rowmin tiles 128x128 blocks via TensorE transpose (identity
          matmul) into PSUM, evacuated to SBUF by the Scalar engine.
  pass 2: sliding-min cascade along H; the last step is scalar_tensor_tensor
          with accum_out, yielding per-partition partial sums directly.
  out:    [128, 1] per-partition sums (reduce over accum columns).
"""

import numpy as np

try:
    import concourse.bass as bass
except ImportError:  # pragma: no cover
    import sys

    sys.path.insert(0, "/opt/trn_rl_repo")
    import concourse.bass as bass

import concourse.mybir as mybir
import concourse.bacc as bacc
from concourse.tile import TileContext
from concourse.bass_utils import run_bass_kernel_spmd

F32 = mybir.dt.float32
BF16 = mybir.dt.bfloat16
INF = float("inf")
MIN = mybir.AluOpType.min

B, C, H, W = 32, 3, 512, 512
WIN = 15
PAD = WIN // 2          # 7
HP = H + 2 * PAD        # 526 padded rows
WP = W + 2 * PAD        # 526 padded cols
N_CORES = 8
N_IMG = B // N_CORES    # 4 images per core
PT = (HP + 127) // 128  # 5 partition tiles of rows / cols
FREE = PT * 128         # 640: free-dim width (526 valid + inf padding)
DEN = B * HP * WP       # element count of the dark-channel map


def build_program(n_img=N_IMG, cast_dma=True):
    nc = bacc.Bacc("TRN2", target_bir_lowering=False, debug=False)
    x = nc.dram_tensor("x", [n_img, C, H, W], F32, kind="ExternalInput")
    out = nc.dram_tensor("out", [128, 1], F32, kind="ExternalOutput")

    n_acc = n_img * PT  # one accum column per (image, W-tile)
    ch_dt = BF16 if cast_dma else F32
    ch_dma = nc.gpsimd if cast_dma else nc.sync

    with TileContext(nc) as tc:
        from contextlib import ExitStack

        with ExitStack() as ctx:
            constp = ctx.enter_context(tc.tile_pool(name="const", bufs=1))
            chp = ctx.enter_context(tc.tile_pool(name="ch", bufs=2))
            tmpp = ctx.enter_context(tc.tile_pool(name="tmp", bufs=2))
            mp = ctx.enter_context(tc.tile_pool(name="m", bufs=2))
            cascp = ctx.enter_context(tc.tile_pool(name="casc", bufs=2))
            rmp = ctx.enter_context(tc.tile_pool(name="rm", bufs=3))
            tbp = ctx.enter_context(tc.tile_pool(name="tb", bufs=2 * PT))
            dcp = ctx.enter_context(tc.tile_pool(name="dc", bufs=2))
            accp = ctx.enter_context(tc.tile_pool(name="acc", bufs=1))
            psp = ctx.enter_context(
                tc.tile_pool(name="ps", bufs=6, space="PSUM")
            )

            ident = constp.tile([128, 128], BF16, tag="ident")
            idt = constp.tile([128, 128], mybir.dt.int16, tag="idt")
            # identity: iota value (col - partition) == 0
            nc.gpsimd.iota(idt[:, :], pattern=[[1, 128]], base=0, channel_multiplier=-1)
            nc.vector.tensor_single_scalar(
                ident[:, :], idt[:, :], 0, mybir.AluOpType.is_equal
            )
            acc = accp.tile([128, n_acc], F32, tag="acc")
            nc.vector.memset(acc[:, :], 0.0)

            for i in range(n_img):
                tb = [
                    tbp.tile([128, FREE], BF16, tag=f"tb{p}", name=f"tb{p}_{i}")
                    for p in range(PT)
                ]

                # ---- load: computed padded rows 7..518 = src rows 0..511,
                # 4 row-tiles of 128, all 3 channels in one DMA ----
                NT = H // 128  # 4
                cht = chp.tile([128, C, NT, W], ch_dt, tag="ch", name=f"ch_{i}")
                ch_dma.dma_start(
                    cht[:, :, :, :],
                    x[i, :, :, :].rearrange("c (q p) w -> p c q w", p=128),
                )
                ch = [cht[:, c] for c in range(C)]

                # ---- pass 1: channel-min + W cascade, per row-tile ----
                for t in range(NT):
                    m = mp.tile([128, FREE], BF16, tag="m", name=f"m_{i}_{t}")
                    nc.gpsimd.memset(m[:, WP:FREE], INF)
                    tmp = tmpp.tile([128, W], ch_dt, tag="tmp", name=f"tmp_{i}_{t}")
                    nc.vector.tensor_tensor(
                        tmp[:, :], ch[0][:, t, :], ch[1][:, t, :], MIN
                    )
                    nc.vector.tensor_tensor(
                        m[:, PAD : PAD + W], tmp[:, :], ch[2][:, t, :], MIN
                    )
                    # reflect pads along W (reversed in-tile copies)
                    nc.vector.tensor_copy(m[:, 0:PAD], m[:, 2 * PAD : PAD : -1])
                    nc.vector.tensor_copy(
                        m[:, W + PAD : WP], m[:, W + PAD - 2 : W - 2 : -1]
                    )

                    w2 = cascp.tile([128, FREE], BF16, tag="w2", name=f"w2_{i}_{t}")
                    w4 = cascp.tile([128, FREE], BF16, tag="w4", name=f"w4_{i}_{t}")
                    w8 = cascp.tile([128, FREE], BF16, tag="w8", name=f"w8_{i}_{t}")
                    n2 = WP + WIN - 2  # 539
                    nc.vector.tensor_tensor(
                        w2[:, 0:n2], m[:, 0:n2], m[:, 1 : n2 + 1], MIN
                    )
                    n4 = n2 - 2
                    nc.vector.tensor_tensor(
                        w4[:, 0:n4], w2[:, 0:n4], w2[:, 2 : n4 + 2], MIN
                    )
                    n8 = n4 - 4
                    nc.vector.tensor_tensor(
                        w8[:, 0:n8], w4[:, 0:n8], w4[:, 4 : n8 + 4], MIN
                    )

                    rm = rmp.tile([128, FREE], BF16, tag="rm", name=f"rm_{i}_{t}")
                    nc.gpsimd.memset(rm[:, WP:FREE], INF)
                    nc.vector.tensor_tensor(
                        rm[:, 0:WP], w8[:, 0:WP], w8[:, PAD : WP + PAD], MIN
                    )

                    # transpose each 128-col block via TensorE -> PSUM -> SBUF;
                    # row-tile t lands at free cols 7+128t .. 7+128(t+1)
                    for p in range(PT):
                        pst = psp.tile([128, 128], BF16, tag="pst", name=f"pst_{i}_{t}_{p}")
                        nc.tensor.transpose(
                            pst[:, :], rm[:, 128 * p : 128 * (p + 1)], ident[:, :]
                        )
                        nc.scalar.copy(
                            tb[p][:, PAD + 128 * t : PAD + 128 * (t + 1)], pst[:, :]
                        )

                # ---- pass 2: H cascade + accumulate ----
                for p in range(PT):
                    wp = min(128, WP - 128 * p)
                    # inf pad beyond the valid padded-H range
                    nc.gpsimd.memset(tb[p][:, WP:FREE], INF)
                    # row reflection on the free dim: padded rows 0..6 <- 14..8,
                    # padded rows 519..525 <- 517..511
                    nc.vector.tensor_copy(
                        tb[p][0:wp, 0:PAD], tb[p][0:wp, 2 * PAD : PAD : -1]
                    )
                    nc.vector.tensor_copy(
                        tb[p][0:wp, H + PAD : HP], tb[p][0:wp, H + PAD - 2 : H - 2 : -1]
                    )
                    h2 = cascp.tile([128, FREE], BF16, tag="h2", name=f"h2_{i}_{p}")
                    h4 = cascp.tile([128, FREE], BF16, tag="h4", name=f"h4_{i}_{p}")
                    h8 = cascp.tile([128, FREE], BF16, tag="h8", name=f"h8_{i}_{p}")
                    n2 = HP + WIN - 2
                    nc.vector.tensor_tensor(
                        h2[0:wp, 0:n2], tb[p][0:wp, 0:n2], tb[p][0:wp, 1 : n2 + 1], MIN
                    )
                    n4 = n2 - 2
                    nc.vector.tensor_tensor(
                        h4[0:wp, 0:n4], h2[0:wp, 0:n4], h2[0:wp, 2 : n4 + 2], MIN
                    )
                    n8 = n4 - 4
                    nc.vector.tensor_tensor(
                        h8[0:wp, 0:n8], h4[0:wp, 0:n8], h4[0:wp, 4 : n8 + 4], MIN
                    )

                    dcs = dcp.tile([128, HP], BF16, tag="dcs", name=f"dcs_{i}_{p}")
                    k = i * PT + p
                    nc.vector.scalar_tensor_tensor(
                        dcs[0:wp, 0:HP],
                        h8[0:wp, 0:HP],
                        0.0,
                        h8[0:wp, PAD : HP + PAD],
                        mybir.AluOpType.bypass,
                        MIN,
                        accum_out=acc[0:wp, k : k + 1],
                    )

            tot = accp.tile([128, 1], F32, tag="tot")
            nc.vector.tensor_reduce(
                tot[:, 0:1],
                acc[:, 0:n_acc],
                axis=mybir.AxisListType.X,
                op=mybir.AluOpType.add,
            )
            nc.sync.dma_start(out[:, :], tot[:, :])

    return nc


_PROGRAM = None


def _get_program():
    global _PROGRAM
    if _PROGRAM is None:
        _PROGRAM = build_program()
        _PROGRAM.finalize()  # run Bacc passes (wait splitting, regalloc)
    return _PROGRAM


def kernel(generated_image):
    x = np.ascontiguousarray(np.asarray(generated_image), dtype=np.float32)
    assert x.shape == (B, C, H, W)
    nc = _get_program()
    shards = x.reshape(N_CORES, N_IMG, C, H, W)
    in_maps = [{"x": np.ascontiguousarray(shards[i])} for i in range(N_CORES)]
    res = run_bass_kernel_spmd(nc, in_maps, list(range(N_CORES)))
    total = float(np.sum([r["out"].astype(np.float64).sum() for r in res.results]))
    return np.array(-total / DEN, dtype=np.float32)
